# revision 28
# baseline (speedup 1.0000x reference)
"""Trainium2 Bass kernel for nn_Agent_86011015070195 (CBF-QP controller).

Computation per row s in R^16 (B*T = 1M rows):
  t1 = tanh(s@Wc1 + bc1); u_unc = 2*(t1@Wc2 + bc2)
  t2 = tanh(s@Wh1 + bh1); h = t2@wh2 + bh2
  dh = ((1-t2^2)*wh2) @ Wh1^T          (= c - W'^T @ t2^2)
  f = s@Wf + bf; g = (s@Wg + bg).reshape(16,4)
  L[a] = sum_s dh_s g[s,a]  (left = -L); dhf = dh.f
  viol = -L.u_unc - (h + dhf); lam = relu(viol)/(sum L^2 + eps)
  u = u_unc + lam*L

Layout: shard B*T across 8 cores (131072 rows each), host-packs each
core's rows into sT8 [128, 16384]: partition 32*(j//2)+16*(j%2)+s holds
feature s of slab j (8 slabs of 16384 rows). Feature-on-partition
matmul pipeline; per-slab scalars packed into 32-partition PSUM strips;
tail reductions batched across slabs.
"""
import json

import numpy as np

import concourse.bass as bass
import concourse.bass2jax as _b2j
import concourse.mybir as mybir
from concourse.tile import TileContext
from concourse.bass_utils import run_bass_kernel_spmd

# ---------------------------------------------------------------------------
# The walrus build in this container caps sync-wait commands at 2 per
# instruction; Tile's kernel-tail drain (and dense first-consumers) can carry
# more. Split excess waits onto preceding same-engine NoOps at the BIR level.
_MAXW = 1


def _split_excess_waits(bir_json: bytes) -> bytes:
    m = json.loads(bir_json)
    changed = False
    for fn in m["functions"]:
        for bb in fn["blocks"]:
            newl = []
            for ins in bb["instructions"]:
                si = ins.get("sync_info")
                ow = (si or {}).get("on_wait") or []
                if len(ow) > _MAXW:
                    chunks = [ow[i:i + _MAXW] for i in range(0, len(ow), _MAXW)]
                    for ci, ch in enumerate(chunks[:-1]):
                        newl.append({
                            "debug": ins.get("debug"),
                            "engine": ins["engine"],
                            "ins": [], "outs": [],
                            "name": f"{ins['name']}_xw{ci}",
                            "opcode": "NoOp",
                            "sync_info": {"on_update": [], "on_wait": ch},
                        })
                    si["on_wait"] = chunks[-1]
                    changed = True
                newl.append(ins)
            bb["instructions"] = newl
    return json.dumps(m).encode() if changed else bir_json


_orig_cbk = _b2j.compile_bir_kernel


def _patched_cbk(bir_json, tmpdir, neff_name="file.neff"):
    return _orig_cbk(_split_excess_waits(bir_json), tmpdir, neff_name)


_b2j.compile_bir_kernel = _patched_cbk

F32 = mybir.dt.float32
MUL = mybir.AluOpType.mult
ADD = mybir.AluOpType.add
AFT = mybir.ActivationFunctionType

B, T, S, H, A = 512, 2048, 16, 128, 4
EPS = 1e-8
NCORES = 8
NPC = B * T // NCORES          # rows per core = 131072
RSL = NPC // 8                 # rows per slab = 16384
NFREE = 512                    # free-dim columns per tile
NT = RSL // NFREE              # tiles = 32

# matmul input dtype: float32 (exact) or float32r (faster PE, ~tf32)
MM_DT = F32


def _build_host_weights(Wc1, bc1, Wc2, bc2, Wh1, bh1, wh2, bh2, Wf, bf, Wg, bg):
    """Build packed lhsT / bias tensors on the host. All fp32."""
    f4 = np.float32

    # --- S-stage weights: W_all [16, 336] = [Wc1 | Wh1 | g_r | Wf]
    W_all = np.zeros((16, 336), f4)
    W_all[:, 0:128] = Wc1
    W_all[:, 128:256] = Wh1
    for a in range(4):
        for so in range(16):
            W_all[:, 256 + a * 16 + so] = Wg[:, so * 4 + a]
    W_all[:, 320:336] = Wf
    # wS8 [128, 672]: pair i rows 32i..32i+16 = even slab weights (cols 0:336),
    # rows 32i+16..32i+32 = odd slab weights (cols 336:672)
    wS8 = np.zeros((128, 672), f4)
    for i in range(4):
        wS8[32 * i:32 * i + 16, 0:336] = W_all
        wS8[32 * i + 16:32 * i + 32, 336:672] = W_all

    # --- H-stage weights wH [128, 144]: u0 [0:32], h [32:64], mdh [64:144]
    wH = np.zeros((128, 144), f4)
    wH[:, 0:4] = 2.0 * Wc2                      # u0 = 2*t1@Wc2
    wH[:, 32 + 5] = wh2                          # h into Rd strip row 5
    Wp = Wh1.T * wh2[:, None]                    # W'[j,s] = Wh1[s,j]*wh2[j]
    for a in range(4):
        wH[:, 64 + a * 16:64 + a * 16 + 16] = -Wp
    wH[:, 64 + 64:64 + 80] = -Wp                 # plain copy rows 64..79

    # --- E-stage (rhs prod [80, n]) wE [80, 32]: L cols 0..3, dhf col 4
    wE = np.zeros((80, 32), f4)
    for a in range(4):
        wE[a * 16:a * 16 + 16, a] = 1.0
    wE[64:80, 4] = 1.0

    # --- vm-red weights [128, 64] x 3 per bank (rhs rsb_b, ppu_b, pLL_b)
    # V rows 0..7 = viol_j, 32..39 = sumsq_j (32-aligned strips)
    wVa = np.zeros((2, 128, 64), f4)
    wVb = np.zeros((2, 128, 64), f4)
    wVc = np.zeros((2, 128, 64), f4)
    for b in range(2):
        for k in range(4):
            j = 4 * b + k
            wVa[b, 32 * k + 4, j] = -1.0         # -dhf
            wVa[b, 32 * k + 5, j] = -1.0         # -h
            wVa[b, 32 * k:32 * k + 4, j] = -2.0 * bc2   # -2*bc2 . L
            wVb[b, 32 * k:32 * k + 4, j] = -1.0  # -sum ppu
            wVc[b, 32 * k:32 * k + 4, 32 + j] = 1.0  # sumsq
    # --- lam broadcast wB [9, 256]: per bank b cols b*128 + 32k+q (q=0..3)
    wB = np.zeros((9, 256), f4)
    for b in range(2):
        for k in range(4):
            j = 4 * b + k
            wB[j, b * 128 + 32 * k:b * 128 + 32 * k + 4] = 1.0

    # --- per-partition constant columns cc [128, 4]
    cc = np.zeros((128, 4), f4)
    cc[:, 0] = bc1
    cc[:, 1] = bh1
    c = Wp.sum(axis=0)                           # c[s] = sum_j W'[j,s]
    for a in range(4):
        cc[a * 16:a * 16 + 16, 2] = c
    cc[64:80, 2] = c
    cc[0:8, 3] = -bh2[0]                         # viol bias (pre-relu)
    cc[32:64, 3] = EPS                           # sumsq + eps (strip 1)

    return {
        "wS8": wS8, "wH": wH, "wE": wE,
        "wVa": np.concatenate([wVa[0], wVa[1]], axis=1),
        "wVb": np.concatenate([wVb[0], wVb[1]], axis=1),
        "wVc": np.concatenate([wVc[0], wVc[1]], axis=1),
        "wB": wB, "cc": cc,
        "bias_fg_nonzero": bool(np.any(bf != 0) or np.any(bg != 0)),
        "bf": bf, "bg": bg,
    }


def build_nc(bias_fg=None, debug=False):
    """Build the Bass program (per-core SPMD; same program all cores)."""
    nc = bass.Bass()
    dbg = {}
    if debug:
        for nm, shp in [("dbg_t1", [128, NFREE]), ("dbg_rd1", [128, NFREE]),
                        ("dbg_u0sb1", [128, NFREE]), ("dbg_V", [64, NFREE]),
                        ("dbg_lam", [9, NFREE]), ("dbg_uu1", [128, NFREE]),
                        ("dbg_gf", [80, NFREE]), ("dbg_prod", [80, NFREE])]:
            dbg[nm] = nc.dram_tensor(nm, shp, F32, kind="ExternalOutput")
    d_sT8 = nc.dram_tensor("sT8", [128, RSL], F32, kind="ExternalInput")
    d_wS8 = nc.dram_tensor("wS8", [128, 672], F32, kind="ExternalInput")
    d_wH = nc.dram_tensor("wH", [128, 144], F32, kind="ExternalInput")
    d_wE = nc.dram_tensor("wE", [80, 32], F32, kind="ExternalInput")
    d_wVa = nc.dram_tensor("wVa", [128, 128], F32, kind="ExternalInput")
    d_wVb = nc.dram_tensor("wVb", [128, 128], F32, kind="ExternalInput")
    d_wVc = nc.dram_tensor("wVc", [128, 128], F32, kind="ExternalInput")
    d_wB = nc.dram_tensor("wB", [9, 256], F32, kind="ExternalInput")
    d_cc = nc.dram_tensor("cc", [128, 4], F32, kind="ExternalInput")
    d_uT = nc.dram_tensor("uT", [32, RSL], F32, kind="ExternalOutput")

    with TileContext(nc) as tc:
        with (
            tc.sbuf_pool(name="wpool", bufs=1) as wp,
            tc.sbuf_pool(name="io", bufs=3) as io,
            tc.sbuf_pool(name="mid", bufs=2) as mid,
            tc.sbuf_pool(name="tail", bufs=2) as tl,
            tc.psum_pool(name="big", bufs=2) as pbig,
            tc.psum_pool(name="aux", bufs=1) as paux,
            tc.psum_pool(name="small", bufs=2) as psml,
            tc.psum_pool(name="rd", bufs=1) as prd,
        ):
            # ---- load weights once
            ws8 = wp.tile([128, 672], MM_DT)
            nc.sync.dma_start(ws8[:], d_wS8[:])
            wh = wp.tile([128, 144], MM_DT)
            nc.sync.dma_start(wh[:], d_wH[:])
            we = wp.tile([80, 32], MM_DT)
            nc.sync.dma_start(we[:], d_wE[:])
            wva = wp.tile([128, 128], MM_DT)
            nc.sync.dma_start(wva[:], d_wVa[:])
            wvb = wp.tile([128, 128], MM_DT)
            nc.sync.dma_start(wvb[:], d_wVb[:])
            wvc = wp.tile([128, 128], MM_DT)
            nc.sync.dma_start(wvc[:], d_wVc[:])
            wb = wp.tile([9, 256], MM_DT)
            nc.sync.dma_start(wb[:], d_wB[:])
            cc = wp.tile([128, 4], F32)
            nc.sync.dma_start(cc[:], d_cc[:])
            lam9 = wp.tile([9, NFREE], F32)
            nc.vector.memset(lam9[:], 1.0)  # row 8 stays 1.0 (ones row)

            for it in range(NT):
                cs = slice(it * NFREE, (it + 1) * NFREE)
                st = io.tile([128, NFREE], MM_DT, tag="st")
                nc.sync.dma_start(st[:], d_sT8[:, cs])

                Rd0 = prd.tile([128, NFREE], F32, tag="rd0")
                Rd1 = prd.tile([128, NFREE], F32, tag="rd1")
                Rds = [Rd0, Rd1]
                u0sb0 = tl.tile([128, NFREE], F32, tag="u0sb0")
                u0sb1 = tl.tile([128, NFREE], F32, tag="u0sb1")
                u0sbs = [u0sb0, u0sb1]

                for j in range(8):
                    i, o = j // 2, j % 2
                    rsl = slice(32 * i, 32 * i + 32)
                    wcol = 336 * o
                    b, k = j // 4, j % 4

                    z1 = pbig.tile([128, NFREE], F32, tag="z")
                    nc.tensor.matmul(z1[:], ws8[rsl, wcol:wcol + 128],
                                     st[rsl, :], tile_position=(32 * i, 0))
                    t1 = mid.tile([128, NFREE], F32, tag="t1")
                    nc.scalar.activation(t1[:], z1[:], AFT.Tanh, bias=cc[:, 0:1])

                    z2 = pbig.tile([128, NFREE], F32, tag="z")
                    nc.tensor.matmul(z2[:], ws8[rsl, wcol + 128:wcol + 256],
                                     st[rsl, :], tile_position=(32 * i, 0))
                    t2 = mid.tile([128, NFREE], F32, tag="t2")
                    nc.scalar.activation(t2[:], z2[:], AFT.Tanh, bias=cc[:, 1:2])

                    sq = mid.tile([128, NFREE], F32, tag="sq")
                    nc.gpsimd.tensor_tensor(sq[:], t2[:], t2[:], MUL)

                    gf = paux.tile([80, NFREE], F32, tag="gf")
                    nc.tensor.matmul(gf[:], ws8[rsl, wcol + 256:wcol + 336],
                                     st[rsl, :], tile_position=(32 * i, 0))

                    mdh = paux.tile([80, NFREE], F32, tag="mdh")
                    nc.tensor.matmul(mdh[:], wh[:, 64:144], sq[:])
                    dh = mid.tile([80, NFREE], F32, tag="dh")
                    nc.vector.tensor_scalar(dh[:], mdh[:], cc[0:80, 2:3],
                                            None, ADD)
                    prod = mid.tile([80, NFREE], F32, tag="prod")
                    nc.vector.tensor_tensor(prod[:], dh[:], gf[:], MUL)

                    # u0 strip -> SBUF
                    u0 = psml.tile([32, NFREE], F32, tag="u0")
                    nc.tensor.matmul(u0[:], wh[:, 0:32], t1[:])
                    nc.scalar.copy(u0sbs[b][32 * k:32 * k + 32, :], u0[:])

                    if debug and it == 0 and j == 5:
                        nc.sync.dma_start(dbg["dbg_t1"][:], t1[:])
                        gfc = tl.tile([80, NFREE], F32, tag="dbgc")
                        nc.vector.tensor_scalar_add(gfc[:], gf[:], 0.0)
                        nc.sync.dma_start(dbg["dbg_gf"][:], gfc[:])
                        nc.sync.dma_start(dbg["dbg_prod"][:], prod[:])

                    # Rd strip: h (from t2) + L/dhf (from prod), accumulated
                    Rd = Rds[b]
                    nc.tensor.matmul(Rd[32 * k:32 * k + 32, :], wh[:, 32:64],
                                     t2[:], start=True, stop=False,
                                     tile_position=(0, 32 * k),
                                     skip_group_check=True)
                    nc.tensor.matmul(Rd[32 * k:32 * k + 32, :], we[:],
                                     prod[:], start=False, stop=True,
                                     tile_position=(0, 32 * k),
                                     skip_group_check=True)

                # ---- tail (packed across slabs)
                V = psml.tile([64, NFREE], F32, tag="u0")
                rsbs, ppus, plls = [], [], []
                for b in range(2):
                    rsb = tl.tile([128, NFREE], F32, tag=f"rsb{b}")
                    nc.vector.tensor_scalar_add(rsb[:], Rds[b][:], 0.0)
                    ppu = tl.tile([128, NFREE], F32, tag=f"ppu{b}")
                    nc.vector.tensor_tensor(ppu[:], u0sbs[b][:], Rds[b][:], MUL)
                    pll = tl.tile([128, NFREE], F32, tag=f"pll{b}")
                    nc.vector.tensor_tensor(pll[:], rsb[:], Rds[b][:], MUL)
                    rsbs.append(rsb)
                    ppus.append(ppu)
                    plls.append(pll)
                for b in range(2):
                    csl = slice(64 * b, 64 * b + 64)
                    nc.tensor.matmul(V[:], wva[:, csl],
                                     rsbs[b][:], start=(b == 0), stop=False,
                                     skip_group_check=True)
                    nc.tensor.matmul(V[:], wvb[:, csl],
                                     ppus[b][:], start=False, stop=False,
                                     skip_group_check=True)
                    nc.tensor.matmul(V[:], wvc[:, csl],
                                     plls[b][:], start=False, stop=(b == 1),
                                     skip_group_check=True)

                if debug and it == 0:
                    nc.sync.dma_start(dbg["dbg_rd1"][:], rsbs[1][:])
                    nc.sync.dma_start(dbg["dbg_u0sb1"][:], u0sbs[1][:])
                    vc = tl.tile([64, NFREE], F32, tag="dbgv")
                    nc.vector.tensor_scalar_add(vc[:], V[:], 0.0)
                    nc.sync.dma_start(dbg["dbg_V"][:], vc[:])

                vrel = tl.tile([32, NFREE], F32, tag="vrel")
                nc.scalar.activation(vrel[:], V[0:32, :], AFT.Relu,
                                     bias=cc[0:32, 3:4])
                vsse = tl.tile([32, NFREE], F32, tag="vsse")
                nc.scalar.activation(vsse[:], V[32:64, :], AFT.Relu,
                                     bias=cc[32:64, 3:4])
                rr = tl.tile([32, NFREE], F32, tag="rr")
                nc.vector.reciprocal(rr[:], vsse[:])
                nc.vector.tensor_tensor(lam9[0:8, :], vrel[0:8, :],
                                        rr[0:8, :], MUL)

                for b in range(2):
                    LB = pbig.tile([128, NFREE], F32, tag="z")
                    nc.tensor.matmul(LB[:], wb[:, 128 * b:128 * b + 128],
                                     lam9[:])
                    tt = tl.tile([128, NFREE], F32, tag=f"tt{b}")
                    nc.vector.tensor_tensor(tt[:], rsbs[b][:], LB[:], MUL)
                    uu = tl.tile([128, NFREE], F32, tag=f"uu{b}")
                    nc.vector.tensor_tensor(uu[:], tt[:], u0sbs[b][:], ADD)
                    for k in range(4):
                        nc.sync.dma_start(
                            d_uT[16 * b + 4 * k:16 * b + 4 * k + 4, cs],
                            uu[32 * k:32 * k + 4, :])
                    if debug and it == 0 and b == 1:
                        nc.sync.dma_start(dbg["dbg_lam"][:], lam9[:])
                        nc.sync.dma_start(dbg["dbg_uu1"][:], uu[:])
    return nc


_CACHED = {}


def _get_nc():
    if "nc" not in _CACHED:
        _CACHED["nc"] = build_nc()
    return _CACHED["nc"]


class PjrtRunner:
    """Compile the Bass program to a PJRT executable once; rerun cheaply.

    Mirrors bass2jax.run_bass_via_pjrt's multi-core branch but caches the
    jitted function so repeated calls don't re-trace/re-compile.
    """

    def __init__(self, nc, n_cores=NCORES):
        import jax
        from jax.sharding import Mesh, PartitionSpec
        from jax.experimental.shard_map import shard_map
        from concourse import bass2jax as b2j

        b2j.install_neuronx_cc_hook()
        self.n_cores = n_cores
        in_names, out_names, out_avals, zero_outs = [], [], [], []
        for alloc in nc.m.functions[0].allocations:
            if not isinstance(alloc, mybir.MemoryLocationSet):
                continue
            name = alloc.memorylocations[0].name
            if alloc.kind == "ExternalInput":
                if nc.partition_id_tensor is None or \
                        name != nc.partition_id_tensor.name:
                    in_names.append(name)
            elif alloc.kind == "ExternalOutput":
                out_names.append(name)
                shape = tuple(alloc.tensor_shape)
                dtype = mybir.dt.np(alloc.dtype)
                out_avals.append(jax.core.ShapedArray(shape, dtype))
                zero_outs.append(np.zeros(shape, dtype))
        self.in_names = list(in_names)
        self.out_names = out_names
        self.out_avals = out_avals
        self.zero_outs = zero_outs
        n_params = len(in_names)
        self.n_params = n_params
        all_in_names = in_names + out_names
        pid_name = None
        if nc.partition_id_tensor is not None:
            pid_name = nc.partition_id_tensor.name
            all_in_names = all_in_names + [pid_name]

        def _body(*args):
            operands = list(args)
            if pid_name is not None:
                operands.append(b2j.partition_id_tensor())
            outs = b2j._bass_exec_p.bind(
                *operands,
                out_avals=tuple(out_avals),
                in_names=tuple(all_in_names),
                out_names=tuple(out_names),
                lowering_input_output_aliases=(),
                sim_require_finite=True,
                sim_require_nnan=True,
                nc=nc,
            )
            return tuple(outs)

        devices = jax.devices()[:n_cores]
        mesh = Mesh(np.asarray(devices), ("core",))
        n_outs = len(out_names)
        self.sharded = jax.jit(
            shard_map(_body, mesh=mesh,
                      in_specs=(PartitionSpec("core"),) * (n_params + n_outs),
                      out_specs=(PartitionSpec("core"),) * n_outs,
                      check_rep=False),
            donate_argnums=tuple(range(n_params, n_params + n_outs)),
            keep_unused=True,
        )

    def prep(self, in_maps):
        return [
            np.concatenate([np.asarray(in_maps[c][nm])
                            for c in range(self.n_cores)], axis=0)
            for nm in self.in_names
        ]

    def __call__(self, concat_in):
        zs = [np.zeros((self.n_cores * z.shape[0], *z.shape[1:]), z.dtype)
              for z in self.zero_outs]
        out_arrs = self.sharded(*concat_in, *zs)
        return out_arrs

    def results(self, out_arrs):
        return [
            {nm: np.asarray(out_arrs[i]).reshape(
                self.n_cores, *self.out_avals[i].shape)[c]
             for i, nm in enumerate(self.out_names)}
            for c in range(self.n_cores)
        ]


def _prep_inputs(inputs):
    """Host-side packing. Returns per-core in_maps and the bc2 for later."""
    f4 = np.float32
    ins = {k: np.asarray(v, f4) for k, v in inputs.items()}
    w = _build_host_weights(
        ins["Wc1"], ins["bc1"], ins["Wc2"], ins["bc2"], ins["Wh1"],
        ins["bh1"], ins["wh2"], ins["bh2"], ins["Wf"], ins["bf"],
        ins["Wg"], ins["bg"])
    assert not w["bias_fg_nonzero"], "nonzero bf/bg not supported in fast path"
    state = ins["state"].reshape(B * T, S)
    shared = {k: np.ascontiguousarray(w[k]) for k in
              ("wS8", "wH", "wE", "wVa", "wVb", "wVc", "wB", "cc")}
    in_maps = []
    for c in range(NCORES):
        sh = state[c * NPC:(c + 1) * NPC]
        sT8 = np.ascontiguousarray(
            sh.reshape(8, RSL, 16).transpose(0, 2, 1).reshape(128, RSL))
        in_maps.append({"sT8": sT8, **shared})
    return in_maps, ins["bc2"]


def _unpack(results, bc2):
    outs = []
    for c in range(NCORES):
        uT = results[c]["uT"]                       # [32, RSL]
        u = uT.reshape(2, 4, 4, RSL).transpose(0, 1, 3, 2).reshape(NPC, 4)
        outs.append(u)
    full = np.concatenate(outs, axis=0) + 2.0 * bc2[None, :]
    return np.ascontiguousarray(full.reshape(B, T, A).astype(np.float32))


def run(inputs, trace=False):
    nc = _get_nc()
    in_maps, bc2 = _prep_inputs(inputs)
    res = run_bass_kernel_spmd(nc, in_maps, core_ids=list(range(NCORES)),
                               trace=trace)
    return _unpack(res.results, bc2), res


def kernel(**inputs) -> np.ndarray:
    out, _ = run(inputs, trace=False)
    return out
